# revision 30
# baseline (speedup 1.0000x reference)
"""Distributed GAT (2-layer, heads=1) on 8 TRN2 NeuronCores — batched V2.

Per-tile batched edge processing: one indirect gather per 128-dst tile
(C chunks of 128 edges, self-loops folded in as chunk 0), one-hot built
per-tile in bf16 (2x DVE mode) in an interleaved layout, softmax weights
folded into the aggregation matmul as a scaled one-hot lhsT, ad[dst] via a
transposed one-hot (tensor_scalar 4x) + C tiny PE matmuls, as[src] shipped
in the gathered bf16 table as a hi/lo compensated pair, attention-vector
projections fused into the GEMMs ([W | W@a_src | W@a_dst]).
"""
import sys
sys.path.insert(0, '/opt/trn_rl_repo')
import numpy as np
import ml_dtypes

import concourse.bass as bass
import concourse.bacc as bacc
import concourse.tile as tile
from concourse import mybir
from concourse.masks import make_identity
from concourse.bass_utils import run_bass_kernel_spmd

N_CORES = 8
N = 100000
NPC = N // N_CORES          # 12500 nodes per core
NT = 98                     # dst tiles per core
NPAD = NT * 128             # 12544 padded nodes per core
NFULL = N_CORES * NPAD      # 100352 padded global nodes
F1, H, O = 256, 64, 20
RB1, RB2 = H + 3, O + 3     # table row: [h | as_hi | as_lo | one]
NEG_SLOPE = 0.2
AF = mybir.ActivationFunctionType
ALU = mybir.AluOpType
AX = mybir.AxisListType
BF = mybir.dt.bfloat16
bf16 = ml_dtypes.bfloat16


def _prep_edges(edge_index):
    """Per-core [128, NT*C] tables with uniform C chunks per dst tile.

    Chunk 0 of each tile holds the 128 self-loop rows; real edges (sorted by
    dst) fill chunks 1..C-1. Pad slots: src index 0 (any valid row), dst
    window 255 (matches no one-hot column -> zero contribution)."""
    src = np.asarray(edge_index[0], dtype=np.int64)
    dst = np.asarray(edge_index[1], dtype=np.int64)

    per_core = []
    maxcnt = 0
    for c in range(N_CORES):
        m = (dst // NPC) == c
        s_c, d_c = src[m], dst[m] % NPC
        order = np.argsort(d_c, kind="stable")
        s_c, d_c = s_c[order], d_c[order]
        sp = (s_c // NPC) * NPAD + (s_c % NPC)
        t_c = d_c // 128
        counts = np.bincount(t_c, minlength=NT)
        maxcnt = max(maxcnt, int(counts.max()))
        per_core.append((sp, d_c, t_c, counts))

    C = 1 + (maxcnt + 127) // 128
    allc = np.stack([pc[3] for pc in per_core])          # [cores, NT]
    Cts = (1 + np.ceil(allc.max(axis=0) / 128).astype(int)).tolist()
    nch = NT * C
    out = []
    for c, (sp, d_c, t_c, counts) in enumerate(per_core):
        srcs = np.zeros((128, nch), dtype=np.int32)
        dwin = np.full((128, nch), 255.0, dtype=np.float32)
        t_idx = np.arange(NT)
        off = t_idx * C
        srcs[:, off] = (c * NPAD + t_idx[None, :] * 128
                        + np.arange(128)[:, None]).astype(np.int32)
        dwin[:, off] = np.arange(128, dtype=np.float32)[:, None]
        start = np.zeros(NT, dtype=np.int64)
        start[1:] = np.cumsum(counts)[:-1]
        i_t = np.arange(len(d_c)) - start[t_c]
        col = t_c * C + 1 + i_t // 128
        row = i_t % 128
        srcs[row, col] = sp.astype(np.int32)
        dwin[row, col] = (d_c - t_c * 128).astype(np.float32)
        # transposed per-chunk dst windows: [1, nch*128], col cg*128+e
        dwinT = np.ascontiguousarray(dwin.T).reshape(1, nch * 128)
        out.append((srcs, dwin.astype(bf16), dwinT.astype(bf16)))
    return out, C, Cts


def _build(C, Cts):
    nch = NT * C
    nc = bacc.Bacc("TRN2", target_bir_lowering=False, debug=False,
                   num_devices=N_CORES)
    dt = mybir.dt.float32
    xT2 = nc.dram_tensor("xT2", [128, NT * 256], dt, kind="ExternalInput")
    w1 = nc.dram_tensor("w1", [F1, H], dt, kind="ExternalInput")
    w1T = nc.dram_tensor("w1T", [H, F1], dt, kind="ExternalInput")
    w2 = nc.dram_tensor("w2", [H, O], dt, kind="ExternalInput")
    w2T = nc.dram_tensor("w2T", [O, H], dt, kind="ExternalInput")
    a1s = nc.dram_tensor("a1s", [H, 1], dt, kind="ExternalInput")
    a1d = nc.dram_tensor("a1d", [H, 1], dt, kind="ExternalInput")
    b1 = nc.dram_tensor("b1", [H], dt, kind="ExternalInput")
    a2s = nc.dram_tensor("a2s", [O, 1], dt, kind="ExternalInput")
    a2d = nc.dram_tensor("a2d", [O, 1], dt, kind="ExternalInput")
    b2 = nc.dram_tensor("b2", [O], dt, kind="ExternalInput")
    srcs = nc.dram_tensor("srcs", [128, nch], mybir.dt.int32, kind="ExternalInput")
    dwind = nc.dram_tensor("dwin", [128, nch], BF, kind="ExternalInput")
    dwinT = nc.dram_tensor("dwinT", [1, nch * 128], BF, kind="ExternalInput")
    iotar = nc.dram_tensor("iotar", [128, 128 * C], BF, kind="ExternalInput")
    pidxd = nc.dram_tensor("pidx", [128, 1], dt, kind="ExternalInput")
    outp = nc.dram_tensor("outp", [NPAD, O], dt, kind="ExternalOutput")

    with tile.TileContext(nc) as tc:
        with tc.tile_pool(name="const", bufs=1) as cp, \
             tc.tile_pool(name="dram", bufs=1, space="DRAM") as dp, \
             tc.tile_pool(name="work", bufs=3) as wp, \
             tc.tile_pool(name="small", bufs=4) as sp_, \
             tc.tile_pool(name="ps", bufs=2, space="PSUM") as pp:

            # ---- constants ----
            w1aE = cp.tile([128, H + 2], dt)
            nc.sync.dma_start(out=w1aE[:, 0:H], in_=w1[0:128, :])
            w1bE = cp.tile([128, H + 2], dt)
            nc.sync.dma_start(out=w1bE[:, 0:H], in_=w1[128:256, :])
            w2ext = cp.tile([H, O + 2], dt)
            nc.sync.dma_start(out=w2ext[:, 0:O], in_=w2[:])
            w1Ta = cp.tile([H, 128], dt); nc.sync.dma_start(out=w1Ta[:], in_=w1T[:, 0:128])
            w1Tb = cp.tile([H, 128], dt); nc.sync.dma_start(out=w1Tb[:], in_=w1T[:, 128:256])
            w2Ts = cp.tile([O, H], dt); nc.sync.dma_start(out=w2Ts[:], in_=w2T[:])
            a1sc = cp.tile([H, 1], dt); nc.sync.dma_start(out=a1sc[:], in_=a1s[:])
            a1dc = cp.tile([H, 1], dt); nc.sync.dma_start(out=a1dc[:], in_=a1d[:])
            a2sc = cp.tile([O, 1], dt); nc.sync.dma_start(out=a2sc[:], in_=a2s[:])
            a2dc = cp.tile([O, 1], dt); nc.sync.dma_start(out=a2dc[:], in_=a2d[:])
            b1_r = cp.tile([128, H], dt)
            nc.sync.dma_start(out=b1_r[:], in_=b1[None, :].to_broadcast([128, H]))
            b2_r = cp.tile([128, O], dt)
            nc.sync.dma_start(out=b2_r[:], in_=b2[None, :].to_broadcast([128, O]))
            iot = cp.tile([128, 128 * C], BF); nc.sync.dma_start(out=iot[:], in_=iotar[:])
            pidx = cp.tile([128, 1], dt); nc.sync.dma_start(out=pidx[:], in_=pidxd[:])
            ident = cp.tile([128, 128], dt); make_identity(nc, ident[:])
            srct = cp.tile([128, nch], mybir.dt.int32)
            nc.sync.dma_start(out=srct[:], in_=srcs[:])
            dwt = cp.tile([128, nch], BF); nc.sync.dma_start(out=dwt[:], in_=dwind[:])
            onec = cp.tile([128, 1], BF); nc.vector.memset(onec[:], 1.0)
            adL1 = cp.tile([128, NT], BF)
            adL2 = cp.tile([128, NT], BF)

            # fused attention-projection columns: v = W @ a  (per K-half)
            vps = pp.tile([128, 4], dt, tag="pad")
            nc.tensor.matmul(out=vps[:, 0:1], lhsT=w1Ta[:], rhs=a1sc[:], start=True, stop=True)
            nc.tensor.matmul(out=vps[:, 1:2], lhsT=w1Tb[:], rhs=a1sc[:], start=True, stop=True)
            nc.tensor.matmul(out=vps[:, 2:3], lhsT=w1Ta[:], rhs=a1dc[:], start=True, stop=True)
            nc.tensor.matmul(out=vps[:, 3:4], lhsT=w1Tb[:], rhs=a1dc[:], start=True, stop=True)
            nc.vector.tensor_copy(out=w1aE[:, H:H + 1], in_=vps[:, 0:1])
            nc.vector.tensor_copy(out=w1bE[:, H:H + 1], in_=vps[:, 1:2])
            nc.vector.tensor_copy(out=w1aE[:, H + 1:H + 2], in_=vps[:, 2:3])
            nc.vector.tensor_copy(out=w1bE[:, H + 1:H + 2], in_=vps[:, 3:4])
            vp2 = pp.tile([H, 2], dt, tag="tr")
            nc.tensor.matmul(out=vp2[:, 0:1], lhsT=w2Ts[:], rhs=a2sc[:], start=True, stop=True)
            nc.tensor.matmul(out=vp2[:, 1:2], lhsT=w2Ts[:], rhs=a2dc[:], start=True, stop=True)
            nc.vector.tensor_copy(out=w2ext[:, O:O + 2], in_=vp2[:])

            # ---- DRAM intermediates ----
            h1comb = dp.tile([NPAD, RB1], BF)
            h1full = dp.tile([NFULL, RB1], BF)
            h2comb = dp.tile([NPAD, RB2], BF)
            h2full = dp.tile([NFULL, RB2], BF)

            # ---- phase 1: h1 = x @ W1ext -> [h | as | ad] ----
            for t in range(NT):
                xt = wp.tile([128, 256], dt, tag="xt")
                nc.sync.dma_start(out=xt[:], in_=xT2[:, t * 256:(t + 1) * 256])
                hp = pp.tile([128, H + 2], dt, tag="pe")
                nc.tensor.matmul(out=hp[:], lhsT=xt[:, 0:128], rhs=w1aE[:], start=True, stop=False)
                nc.tensor.matmul(out=hp[:], lhsT=xt[:, 128:256], rhs=w1bE[:], start=False, stop=True)
                h1c = wp.tile([128, RB1], BF, tag="h1c")
                nc.vector.tensor_copy(out=h1c[:, 0:H], in_=hp[:, 0:H])
                nc.vector.tensor_copy(out=h1c[:, H:H + 1], in_=hp[:, H:H + 1])
                nc.vector.tensor_tensor(out=h1c[:, H + 1:H + 2], in0=hp[:, H:H + 1],
                                        in1=h1c[:, H:H + 1], op=ALU.subtract)
                nc.vector.tensor_copy(out=h1c[:, H + 2:H + 3], in_=onec[:])
                nc.vector.tensor_copy(out=adL1[:, t:t + 1], in_=hp[:, H + 1:H + 2])
                nc.sync.dma_start(out=h1comb[t * 128:(t + 1) * 128, :], in_=h1c[:])

            # ---- phase 2: all-gather layer-1 table ----
            nc.gpsimd.collective_compute(
                "AllGather", ALU.bypass, replica_groups=[list(range(N_CORES))],
                ins=[h1comb.opt()], outs=[h1full.opt()])

            def edge_layer(full_tbl, adL, FW, w_next, bias_r, last):
                RB = FW + 3
                for t in range(NT):
                    Ct = Cts[t]
                    gt = wp.tile([128, C * RB], BF, tag="gt")
                    for k in range(Ct):
                        nc.gpsimd.indirect_dma_start(
                            out=gt[:, k * RB:(k + 1) * RB],
                            out_offset=None, in_=full_tbl[:],
                            in_offset=bass.IndirectOffsetOnAxis(
                                ap=srct[:, t * C + k:t * C + k + 1], axis=0))
                    dwT = wp.tile([128, C * 128], BF, tag="dwT")
                    nc.scalar.dma_start(
                        out=dwT[:, 0:Ct * 128],
                        in_=dwinT[0:1, t * C * 128:t * C * 128 + Ct * 128]
                        .to_broadcast([128, Ct * 128]))
                    # transposed one-hot [d_part, k*128+e] for ad extraction
                    ohT = wp.tile([128, C * 128], BF, tag="ohT")
                    nc.vector.tensor_scalar(
                        out=ohT[:, 0:Ct * 128], in0=dwT[:, 0:Ct * 128],
                        scalar1=pidx[:, 0:1], scalar2=None, op0=ALU.is_equal)
                    ps_ad = pp.tile([128, C], dt, tag="pad")
                    for k in range(Ct):
                        nc.tensor.matmul(out=ps_ad[:, k:k + 1],
                                         lhsT=ohT[:, k * 128:(k + 1) * 128],
                                         rhs=adL[:, t:t + 1], start=True, stop=True)
                    # scores s = as_hi + as_lo + ad   (all [128, C])
                    gt3 = gt[:].rearrange("p (c r) -> p c r", r=RB)
                    s = sp_.tile([128, C], dt, tag="s")
                    nc.vector.tensor_tensor(out=s[:, 0:Ct], in0=ps_ad[:, 0:Ct],
                                            in1=gt3[:, 0:Ct, FW:FW + 1].squeeze(2), op=ALU.add)
                    nc.vector.tensor_tensor(out=s[:, 0:Ct], in0=s[:, 0:Ct],
                                            in1=gt3[:, 0:Ct, FW + 1:FW + 2].squeeze(2), op=ALU.add)
                    ex = sp_.tile([128, C], BF, tag="ex")
                    s2 = sp_.tile([128, C], dt, tag="s2")
                    nc.vector.tensor_scalar(out=s2[:, 0:Ct], in0=s[:, 0:Ct], scalar1=NEG_SLOPE,
                                            scalar2=None, op0=ALU.mult)
                    nc.vector.tensor_tensor(out=s[:, 0:Ct], in0=s[:, 0:Ct],
                                            in1=s2[:, 0:Ct], op=ALU.max)
                    nc.scalar.activation(out=ex[:, 0:Ct], in_=s[:, 0:Ct], func=AF.Exp)
                    # interleaved one-hot [e, j*C+k] and its ex-scaled version
                    oh = wp.tile([128, 128 * C], BF, tag="oh")
                    nc.vector.tensor_tensor(
                        out=oh[:].rearrange("p (j k) -> p j k", k=C)[:, :, 0:Ct],
                        in0=dwt[:, t * C:t * C + Ct].unsqueeze(1).to_broadcast([128, 128, Ct]),
                        in1=iot[:].rearrange("p (j k) -> p j k", k=C)[:, :, 0:Ct],
                        op=ALU.is_equal)
                    sc = wp.tile([128, 128 * C], BF, tag="sc")
                    nc.vector.tensor_tensor(
                        out=sc[:].rearrange("p (j k) -> p j k", k=C)[:, :, 0:Ct],
                        in0=oh[:].rearrange("p (j k) -> p j k", k=C)[:, :, 0:Ct],
                        in1=ex[:, 0:Ct].unsqueeze(1).to_broadcast([128, 128, Ct]),
                        op=ALU.mult)
                    ps = pp.tile([128, RB], dt, tag="pe")
                    for k in range(Ct):
                        nc.tensor.matmul(out=ps[:], lhsT=sc[:, k::C],
                                         rhs=gt[:, k * RB:(k + 1) * RB],
                                         start=(k == 0), stop=(k == Ct - 1))
                    rec = sp_.tile([128, 1], dt, tag="rec")
                    nc.vector.reciprocal(out=rec[:], in_=ps[:, FW + 2:FW + 3])
                    o1 = wp.tile([128, FW], dt, tag="o1")
                    nc.vector.tensor_scalar(out=o1[:], in0=ps[:, 0:FW], scalar1=rec[:, 0:1],
                                            scalar2=None, op0=ALU.mult)
                    nc.vector.tensor_add(out=o1[:], in0=o1[:], in1=bias_r[:])
                    if last:
                        nc.sync.dma_start(out=outp[t * 128:(t + 1) * 128, :], in_=o1[:])
                        continue
                    nc.scalar.activation(out=o1[:], in_=o1[:], func=AF.Relu)
                    trp = pp.tile([FW, 128], dt, tag="tr")
                    nc.tensor.transpose(out=trp[:], in_=o1[:], identity=ident[:])
                    o1T = wp.tile([FW, 128], dt, tag="o1T")
                    nc.vector.tensor_copy(out=o1T[:], in_=trp[:])
                    h2p = pp.tile([128, O + 2], dt, tag="h2")
                    nc.tensor.matmul(out=h2p[:], lhsT=o1T[:], rhs=w_next[:],
                                     start=True, stop=True)
                    h2c = wp.tile([128, RB2], BF, tag="h2c")
                    nc.vector.tensor_copy(out=h2c[:, 0:O], in_=h2p[:, 0:O])
                    nc.vector.tensor_copy(out=h2c[:, O:O + 1], in_=h2p[:, O:O + 1])
                    nc.vector.tensor_tensor(out=h2c[:, O + 1:O + 2], in0=h2p[:, O:O + 1],
                                            in1=h2c[:, O:O + 1], op=ALU.subtract)
                    nc.vector.tensor_copy(out=h2c[:, O + 2:O + 3], in_=onec[:])
                    nc.vector.tensor_copy(out=adL2[:, t:t + 1], in_=h2p[:, O + 1:O + 2])
                    nc.sync.dma_start(out=h2comb[t * 128:(t + 1) * 128, :], in_=h2c[:])

            # ---- phase 3: edge layer 1 (+ fused layer-2 GEMM) ----
            edge_layer(h1full, adL1, H, w2ext, b1_r, last=False)

            # ---- phase 4: all-gather layer-2 table ----
            nc.gpsimd.collective_compute(
                "AllGather", ALU.bypass, replica_groups=[list(range(N_CORES))],
                ins=[h2comb.opt()], outs=[h2full.opt()])

            # ---- phase 5: edge layer 2 ----
            edge_layer(h2full, adL2, O, None, b2_r, last=True)

    nc.compile()
    return nc


def kernel(x, edge_index, W1, a1_src, a1_dst, b1, W2, a2_src, a2_dst, b2):
    x = np.asarray(x, dtype=np.float32)
    edge_arrays, C, Cts = _prep_edges(np.asarray(edge_index))
    nc = _build(C, Cts)

    iota_rep = np.repeat(np.arange(128, dtype=np.float32), C).reshape(1, 128 * C)
    iota_rep = np.broadcast_to(iota_rep, (128, 128 * C)).astype(bf16)
    W1 = np.asarray(W1, np.float32)
    W2 = np.asarray(W2, np.float32)
    common = dict(
        w1=W1, w1T=np.ascontiguousarray(W1.T),
        w2=W2, w2T=np.ascontiguousarray(W2.T),
        a1s=np.asarray(a1_src, np.float32).reshape(H, 1),
        a1d=np.asarray(a1_dst, np.float32).reshape(H, 1),
        b1=np.asarray(b1, np.float32), b2=np.asarray(b2, np.float32),
        a2s=np.asarray(a2_src, np.float32).reshape(O, 1),
        a2d=np.asarray(a2_dst, np.float32).reshape(O, 1),
        iotar=np.ascontiguousarray(iota_rep),
        pidx=np.arange(128, dtype=np.float32).reshape(128, 1),
    )
    in_maps = []
    for c in range(N_CORES):
        srcs, dwin, dwinT = edge_arrays[c]
        xT = np.zeros((256, NPAD), np.float32)
        xT[:, :NPC] = x[c * NPC:(c + 1) * NPC].T
        xT2 = np.concatenate(
            [xT[0:128].reshape(128, NT, 128), xT[128:256].reshape(128, NT, 128)],
            axis=2).reshape(128, NT * 256)
        in_maps.append(dict(common, xT2=np.ascontiguousarray(xT2), srcs=srcs,
                            dwin=dwin, dwinT=dwinT))

    global _LAST_NC, _LAST_INMAPS
    _LAST_NC, _LAST_INMAPS = nc, in_maps
    res = run_bass_kernel_spmd(nc, in_maps, core_ids=list(range(N_CORES)))
    out = np.concatenate(
        [res.results[c]["outp"][:NPC] for c in range(N_CORES)], axis=0)
    return out.astype(np.float32)
